# revision 9
# baseline (speedup 1.0000x reference)
"""DSAFT rank-loss kernel for 8 Trainium2 NeuronCores (Bass/Tile).

loss = (1/n^2) * sum_{i,j} relu(e_j - e_i) * events_i
       + ALPHA * sum(e^2)/n + BETA * sum(log_h^2)/n
with e = log(durations + EPS) - log_h, n = 16384.

Algorithm (quantized staircase, O(n*B)):
  For an increasing edge grid t_1..t_B with per-edge weights w_k
  (midpoint gaps), relu(e_j - e_i) ~= sum_k w_k 1[e_i < t_k <= e_j], so
     pair ~= sum_k w_k * C_k * (Ev - D_k)
  with C_k = #{j : e_j >= t_k}, D_k = sum_i ev_i 1[e_i >= t_k],
  Ev = sum_i ev_i.  B=63 edges + one sentinel edge at -6e4 whose D
  column recovers Ev (and whose weight is 0).  Offline validation:
  rel err <= 1.8e-3 worst-of-13 draws (gate 2e-2).

Sharding: rows (j) are split across the 8 cores -- each core holds its
2048 elements (16 j-blocks of 128) and the full 64-slot edge vector,
computes partial C/D via PE, and the host sums the 8 partial [64,2]
vectors before the final O(B) combine.

Per-core pipeline (TimelineSim-costed):
  - head (~2.3us fixed): dl=[durs|logh] f32 via SP HWDGE DMA;
    ee=[edges|evs] f16 via Pool SWDGE DMA; act-table preload, memsets,
    scatter-index iota, and an out-zeroing DMA all overlap the head.
  - e = Ln(durs+EPS) - logh (Act then DVE).
  - 16 compare tiles g_c[p,k] = (t_k <= e_p) f16, split DVE(12)/Pool(4)
    (DVE 4x mode ~77ns/tile).  Each g is the matmul STATIONARY;
    moving is [ev_c, ones] [128,2], accumulating psum[64,2] = [D|C]
    per edge -- PE cost is 2 cycles/block (cost = moving free size).
  - penalties via Act Square accum rows + one tiny f32 matmul.
  - epilogue: 2 PSUM->SBUF copies, then a PREPARED SWDGE scatter-add
    fires via trigger_dma (tail ~1.0us instead of ~2.2us HWDGE).
"""

import os

import numpy as np

N = 16384
P = 128            # partitions / j's per block
CB = 16            # j-blocks per core (N / NCORES / P)
NCORES = 8
ALPHA = 0.001
BETA = 0.001
EPS = 1e-32

# staircase quantization: NB slots = 1 sentinel + B_REAL real edges
NB = int(os.environ.get("KERN_NB", "64"))
B_REAL = NB - 1
T0 = -16.0
T1 = 6.0
SENTINEL = -60000.0

# Tuning knobs
NPOOL = int(os.environ.get("KERN_NPOOL", "4"))   # j-blocks on the Pool stream
OUT_MODE = os.environ.get("KERN_OUT", "kvwb")  # kvwb | plain

_prog_cache = {}
last_results = None  # BassKernelResults of the most recent run (for profiling)


def _edges_f64():
    """Real edge positions (f16-snapped), as float64."""
    w = (T1 - T0) / B_REAL
    t = T0 + (np.arange(1, B_REAL + 1, dtype=np.float64) - 0.5) * w
    return t.astype(np.float16).astype(np.float64)


def _edge_weights():
    """Host-side per-slot weights: w[0]=0 (sentinel), midpoint gaps else."""
    t = _edges_f64()
    w = np.empty(NB, dtype=np.float64)
    w[0] = 0.0
    wr = np.empty(B_REAL, dtype=np.float64)
    if B_REAL > 1:
        wr[1:-1] = (t[2:] - t[:-2]) / 2.0
        wr[0] = t[1] - t[0]
        wr[-1] = t[-1] - t[-2]
    else:
        wr[0] = (T1 - T0)
    w[1:] = wr
    return w


def _fix_writeback_sync(nc, stage_writers, trigger_name):
    """Re-plumb the PREPARE_ONLY writeback's synchronization.

    Tile's model for a prepared SWDGE writeback assumes double-buffering:
    writers of the prep's source that come after the prep must wait for the
    DMA (a WAR wait on the prep's DMASW lane), and the trigger is unordered
    against them.  This kernel fills the staging tile after the prep and
    fires the trigger last, so that model (a) deadlocks -- the lane sem is
    never incremented for prepared DMAs -- and (b) leaves the trigger racing
    the stage writers on real hardware.  Rewrite at the BIR level, with the
    same sem encoding Tile itself emits:

      1. strip the dead DMASW-lane WAR waits from the stage writers;
      2. make the trigger wait on each stage writer's engine-lane sem at
         its absolute post-write count (write -> DMA read ordering);
      3. repoint remaining waits on the dead lane sem (end-of-program
         drains) at the descriptor's real completion sem (swdge_out >= 16).
    """
    import bass_rust

    all_ins = [i for bb in nc.m.functions[0].blocks for i in bb.instructions]
    my_sem_id = None
    updaters = set()
    for ins in all_ins:
        si = ins.sync_info
        if si is None:
            continue
        for u in si.on_update:
            updaters.add(u.id)
            if (u.ant_name or "") == "swdge_out":
                my_sem_id = u.id
    assert my_sem_id is not None
    repointed = [False]

    # absolute lane-sem value after each stage writer completes
    sem_counts = {}
    writer_waits = []
    for ins in all_ins:
        si = ins.sync_info
        if si is None:
            continue
        for u in si.on_update:
            if u.update_mode == "sem-inc" and isinstance(u.update_value, int):
                sem_counts[u.id] = sem_counts.get(u.id, 0) + u.update_value
                if ins.name in stage_writers:
                    writer_waits.append(
                        (u.id, u.ant_name, sem_counts[u.id]))
    assert len(writer_waits) >= len(stage_writers)

    for ins in all_ins:
        si = ins.sync_info
        if ins.name == trigger_name:
            si = si or bass_rust.SyncInfo(on_wait=[], on_update=[])
            waits = list(si.on_wait)
            for sid, sname, val in writer_waits:
                waits.append(bass_rust.SyncWait(
                    sync_type="semaphore", id=sid, ant_name=sname,
                    wait_mode="sem-ge-imm", wait_value=val, wait_reg=None))
            ins.sync_info = bass_rust.SyncInfo(
                on_wait=waits, on_update=si.on_update)
            continue
        if si is None:
            continue
        if ins.name in stage_writers:
            keep = [w for w in si.on_wait
                    if not ((w.ant_name or "").startswith("DMASW")
                            and w.id not in updaters)]
            ins.sync_info = bass_rust.SyncInfo(
                on_wait=keep, on_update=si.on_update)
            continue
        dead = [w for w in si.on_wait
                if (w.ant_name or "").startswith("DMASW")
                and w.id not in updaters]
        if not dead:
            continue
        new_waits = []
        for w in si.on_wait:
            if w not in dead:
                new_waits.append(w)
            elif not repointed[0]:
                # one drain gates on the writeback's completion sem; the
                # closing all-engine barrier orders everything after it.
                repointed[0] = True
                new_waits.append(bass_rust.SyncWait(
                    sync_type=w.sync_type, id=my_sem_id,
                    ant_name="swdge_out", wait_mode=w.wait_mode,
                    wait_value=16, wait_reg=None))
        ins.sync_info = bass_rust.SyncInfo(
            on_wait=new_waits, on_update=si.on_update)


def _build_program():
    import concourse.bass as bass
    import concourse.bacc as bacc
    import concourse.mybir as mybir
    from concourse.mybir import AluOpType
    from concourse.tile import TileContext
    from contextlib import ExitStack

    f32 = mybir.dt.float32
    f16 = mybir.dt.float16
    i32 = mybir.dt.int32
    AF = mybir.ActivationFunctionType

    NDVE = CB - NPOOL

    nc = bacc.Bacc("TRN2", debug=False)

    # dl: durs | logh (f32); ee: edges | evs (f16)
    dl = nc.dram_tensor("dl", [P, 2 * CB], f32, kind="ExternalInput").ap()
    ee = nc.dram_tensor("ee", [P, NB + CB], f16, kind="ExternalInput").ap()
    out = nc.dram_tensor("out", [1, P, 1, 64], f32, kind="ExternalOutput").ap()

    with TileContext(nc) as tc, ExitStack() as ctx:
        sg = ctx.enter_context(tc.tile_pool(name="sg", bufs=1))
        dve_pool = ctx.enter_context(tc.tile_pool(name="dve_pool", bufs=NDVE))
        gp_pool = ctx.enter_context(tc.tile_pool(name="gp_pool", bufs=max(NPOOL, 1)))
        psums = ctx.enter_context(tc.tile_pool(name="psums", bufs=1, space="PSUM"))

        # ---- early, data-independent work ----
        eps_sb = sg.tile([P, 1], f32, tag="eps_sb")
        nc.vector.memset(eps_sb[:], EPS)
        onesF = sg.tile([P, 1], f32, tag="onesF")
        nc.vector.memset(onesF[:], 1.0)
        st_tile = sg.tile([P, 1, 1, 64], f32, tag="out_sb")
        stage = st_tile[:]
        evones = sg.tile([P, 2, CB], f16, tag="evones")
        nc.vector.memset(evones[:, 1, :], 1.0)
        # fire the activation-table load (natural_log set: Ln + Square)
        dummy = sg.tile([P, 1], f32, tag="dummy")
        nc.scalar.activation(dummy[:], eps_sb[:], AF.Ln)

        # ---- inputs ----
        dl_sb = sg.tile([P, 2 * CB], f32, tag="dl_sb")
        nc.sync.dma_start(out=dl_sb[:], in_=dl)

        if OUT_MODE == "kvwb":
            ctxz = sg.tile([P, 1], i32, tag="ctxz")
            nc.gpsimd.memset(ctxz[:], 0)
        ee_sb = sg.tile([P, NB + CB], f16, tag="ee_sb")
        nc.gpsimd.dma_start(out=ee_sb[:], in_=ee)

        if OUT_MODE == "kvwb":
            # prep the output-writeback descriptors during the input head;
            # kv_writeback fully overwrites the [128,64] out dram region, so
            # no zero pass is needed.  trigger_dma fires it at the end (the
            # trigger carries the RAW edge on the stage writers).
            dma_sem = nc.alloc_semaphore("swdge_out")
            nc.gpsimd.kv_writeback(
                out, stage, ctxz[:], prepare_only=True, sem=dma_sem
            )

        edges_sb = ee_sb[:, 0:NB]
        evs_sb = ee_sb[:, NB : NB + CB]
        durs_sb = dl_sb[:, 0:CB]
        logh_sb = dl_sb[:, CB : 2 * CB]

        # ---- lnd = ln(durs + EPS); e (for penalties) on Pool ----
        lnd = sg.tile([P, CB], f32, tag="lnd")
        nc.scalar.activation(lnd[:], durs_sb, AF.Ln, bias=eps_sb[:])
        e_sb = sg.tile([P, CB], f32, tag="e_sb")
        nc.gpsimd.tensor_tensor(e_sb[:], lnd[:], logh_sb, AluOpType.subtract)

        # ---- stationary [ev, ones] pairs (moving operand per j-block) ----
        nc.gpsimd.tensor_copy(evones[:, 0, :], evs_sb)

        # ---- penalties: Act Square accum rows -> one tiny f32 matmul
        # (lands in column 2 of the shared psum tile so a single epilogue
        # copy stages everything; issued on PE before the chain).
        pen2 = sg.tile([P, 2], f32, tag="pen2")
        pen_scr = sg.tile([P, CB], f32, tag="pen_scr")
        nc.scalar.activation(pen_scr[:], e_sb[:], AF.Square, accum_out=pen2[:, 0:1])
        nc.scalar.activation(pen_scr[:], logh_sb, AF.Square, accum_out=pen2[:, 1:2])
        psum_m = psums.tile([NB, 3], f32, tag="psum_m")
        nc.tensor.matmul(psum_m[0:2, 2:3], pen2[:], onesF[:], start=True, stop=True)

        # ---- main loop: fused compare g_c[p,k] = 1[t_k + h_p <= lnd_p]
        # (== t_k <= e_p), so compares need only lnd + logh, skipping the
        # subtract on the critical path; g is the matmul stationary.
        pool_pos = set()
        if NPOOL > 0:
            stride = CB / NPOOL
            pool_pos = {min(CB - 2, int(stride * i + 1)) for i in range(NPOOL)}
            while len(pool_pos) < NPOOL:  # collision fallback
                pool_pos.add(max(0, CB - 2 - len(pool_pos)))
        for c in range(CB):
            eng = nc.gpsimd if c in pool_pos else nc.vector
            pool = gp_pool if c in pool_pos else dve_pool
            g = pool.tile([P, NB], f16, tag="g")
            eng.tensor_scalar(
                g[:], edges_sb, logh_sb[:, c : c + 1], lnd[:, c : c + 1],
                AluOpType.add, AluOpType.is_le,
            )
            mov = evones[:, :, c : c + 1].rearrange("p a b -> p (a b)")
            nc.tensor.matmul(
                psum_m[:, 0:2], g[:], mov,
                start=(c == 0), stop=(c == CB - 1),
            )

        # ---- epilogue: stage [D|C|pens], then fire the writeback ----
        w2 = nc.vector.tensor_copy(stage[0:NB, 0, 0, 0:3], psum_m[:])
        stage_writers = [w2.ins.name]
        if OUT_MODE == "kvwb":
            trigger_name = nc.gpsimd.trigger_dma(count=None).ins.name
        else:
            nc.sync.dma_start(
                out=out[0, 0:NB, 0, :], in_=stage[0:NB, 0, 0, :]
            )

    if OUT_MODE == "kvwb":
        _fix_writeback_sync(nc, stage_writers, trigger_name)

    nc.compile()
    return nc


def _get_program():
    key = (NB, NPOOL, OUT_MODE)
    if key not in _prog_cache:
        _prog_cache[key] = _build_program()
    return _prog_cache[key]


def _make_in_maps(log_h, durations, events):
    log_h = np.ascontiguousarray(np.asarray(log_h, dtype=np.float32)).reshape(N)
    durations = np.ascontiguousarray(np.asarray(durations, dtype=np.float32)).reshape(N)
    events = np.ascontiguousarray(np.asarray(events, dtype=np.float32)).reshape(N)

    edges16 = np.empty(NB, dtype=np.float16)
    edges16[0] = SENTINEL
    edges16[1:] = _edges_f64().astype(np.float16)

    SL = N // NCORES
    in_maps = []
    for k in range(NCORES):
        sl = slice(k * SL, (k + 1) * SL)
        dl_np = np.empty((P, 2 * CB), dtype=np.float32)
        dl_np[:, 0:CB] = durations[sl].reshape(P, CB)
        dl_np[:, CB : 2 * CB] = log_h[sl].reshape(P, CB)
        ee_np = np.empty((P, NB + CB), dtype=np.float16)
        ee_np[:, 0:NB] = edges16[None, :]
        ee_np[:, NB : NB + CB] = events[sl].reshape(P, CB).astype(np.float16)
        in_maps.append({"dl": dl_np, "ee": ee_np})
    return in_maps


def kernel(log_h, durations, events):
    global last_results
    from concourse import bass_utils

    nc = _get_program()
    in_maps = _make_in_maps(log_h, durations, events)
    res = bass_utils.run_bass_kernel_spmd(
        nc, in_maps, core_ids=list(range(NCORES))
    )
    last_results = res

    D = np.zeros(NB, dtype=np.float64)
    C = np.zeros(NB, dtype=np.float64)
    e2 = 0.0
    lh2 = 0.0
    for k in range(NCORES):
        A = np.asarray(res.results[k]["out"], dtype=np.float64).reshape(P, 64)[:NB]
        D += A[:, 0]
        C += A[:, 1]
        e2 += A[0, 2]
        lh2 += A[1, 2]

    Ev = D[0]  # sentinel column: every e >= -6e4
    w = _edge_weights()
    pair = float(np.sum(w * C * (Ev - D)))
    loss = pair / float(N) ** 2 + ALPHA * e2 / N + BETA * lh2 / N
    return np.float32(loss)


# revision 10
# speedup vs baseline: 1.0744x; 1.0744x over previous
"""DSAFT rank-loss kernel for 8 Trainium2 NeuronCores (Bass/Tile).

loss = (1/n^2) * sum_{i,j} relu(e_j - e_i) * events_i
       + ALPHA * sum(e^2)/n + BETA * sum(log_h^2)/n
with e = log(durations + EPS) - log_h, n = 16384.

Algorithm (quantized staircase, O(n*B)):
  For an increasing edge grid t_1..t_B with per-edge weights w_k
  (midpoint gaps), relu(e_j - e_i) ~= sum_k w_k 1[e_i < t_k <= e_j], so
     pair ~= sum_k w_k * C_k * (Ev - D_k)
  with C_k = #{j : e_j >= t_k}, D_k = sum_i ev_i 1[e_i >= t_k],
  Ev = sum_i ev_i.  B=63 edges + one sentinel edge at -6e4 whose D
  column recovers Ev (and whose weight is 0).  Offline validation:
  rel err <= 1.8e-3 worst-of-13 draws (gate 2e-2).

Sharding: rows (j) are split across the 8 cores -- each core holds its
2048 elements (16 j-blocks of 128) and the full 64-slot edge vector,
computes partial C/D via PE, and the host sums the 8 partial [64,2]
vectors before the final O(B) combine.

Per-core pipeline (TimelineSim-costed):
  - head (~2.3us fixed): dl=[durs|logh] f32 via SP HWDGE DMA;
    ee=[edges|evs] f16 via Pool SWDGE DMA; act-table preload, memsets,
    scatter-index iota, and an out-zeroing DMA all overlap the head.
  - e = Ln(durs+EPS) - logh (Act then DVE).
  - 16 compare tiles g_c[p,k] = (t_k <= e_p) f16, split DVE(12)/Pool(4)
    (DVE 4x mode ~77ns/tile).  Each g is the matmul STATIONARY;
    moving is [ev_c, ones] [128,2], accumulating psum[64,2] = [D|C]
    per edge -- PE cost is 2 cycles/block (cost = moving free size).
  - penalties via Act Square accum rows + one tiny f32 matmul.
  - epilogue: 2 PSUM->SBUF copies, then a PREPARED SWDGE scatter-add
    fires via trigger_dma (tail ~1.0us instead of ~2.2us HWDGE).
"""

import os

import numpy as np

N = 16384
P = 128            # partitions / j's per block
CB = 16            # j-blocks per core (N / NCORES / P)
NCORES = 8
ALPHA = 0.001
BETA = 0.001
EPS = 1e-32

# staircase quantization: NB slots = 1 sentinel + B_REAL real edges
NB = int(os.environ.get("KERN_NB", "64"))
B_REAL = NB - 1
T0 = -16.0
T1 = 6.0
SENTINEL = -60000.0

# Tuning knobs
NPOOL = int(os.environ.get("KERN_NPOOL", "4"))   # j-blocks on the Pool stream
OUT_MODE = os.environ.get("KERN_OUT", "kvwb")  # kvwb | plain

_prog_cache = {}
last_results = None  # BassKernelResults of the most recent run (for profiling)


def _edges_f64():
    """Real edge positions (f16-snapped), as float64."""
    w = (T1 - T0) / B_REAL
    t = T0 + (np.arange(1, B_REAL + 1, dtype=np.float64) - 0.5) * w
    return t.astype(np.float16).astype(np.float64)


def _edge_weights():
    """Host-side per-slot weights: w[0]=0 (sentinel), midpoint gaps else."""
    t = _edges_f64()
    w = np.empty(NB, dtype=np.float64)
    w[0] = 0.0
    wr = np.empty(B_REAL, dtype=np.float64)
    if B_REAL > 1:
        wr[1:-1] = (t[2:] - t[:-2]) / 2.0
        wr[0] = t[1] - t[0]
        wr[-1] = t[-1] - t[-2]
    else:
        wr[0] = (T1 - T0)
    w[1:] = wr
    return w


def _fix_writeback_sync(nc, stage_writers, trigger_name):
    """Re-plumb the PREPARE_ONLY writeback's synchronization.

    Tile's model for a prepared SWDGE writeback assumes double-buffering:
    writers of the prep's source that come after the prep must wait for the
    DMA (a WAR wait on the prep's DMASW lane), and the trigger is unordered
    against them.  This kernel fills the staging tile after the prep and
    fires the trigger last, so that model (a) deadlocks -- the lane sem is
    never incremented for prepared DMAs -- and (b) leaves the trigger racing
    the stage writers on real hardware.  Rewrite at the BIR level, with the
    same sem encoding Tile itself emits:

      1. strip the dead DMASW-lane WAR waits from the stage writers;
      2. make the trigger wait on each stage writer's engine-lane sem at
         its absolute post-write count (write -> DMA read ordering);
      3. repoint remaining waits on the dead lane sem (end-of-program
         drains) at the descriptor's real completion sem (swdge_out >= 16).
    """
    import bass_rust

    all_ins = [i for bb in nc.m.functions[0].blocks for i in bb.instructions]
    my_sem_id = None
    updaters = set()
    for ins in all_ins:
        si = ins.sync_info
        if si is None:
            continue
        for u in si.on_update:
            updaters.add(u.id)
            if (u.ant_name or "") == "swdge_out":
                my_sem_id = u.id
    assert my_sem_id is not None

    # absolute lane-sem value after each stage writer completes
    sem_counts = {}
    writer_waits = []
    for ins in all_ins:
        si = ins.sync_info
        if si is None:
            continue
        for u in si.on_update:
            if u.update_mode == "sem-inc" and isinstance(u.update_value, int):
                sem_counts[u.id] = sem_counts.get(u.id, 0) + u.update_value
                if ins.name in stage_writers:
                    writer_waits.append(
                        (u.id, u.ant_name, sem_counts[u.id]))
    assert len(writer_waits) >= len(stage_writers)

    for ins in all_ins:
        si = ins.sync_info
        if ins.name == trigger_name:
            si = si or bass_rust.SyncInfo(on_wait=[], on_update=[])
            waits = list(si.on_wait)
            for sid, sname, val in writer_waits:
                waits.append(bass_rust.SyncWait(
                    sync_type="semaphore", id=sid, ant_name=sname,
                    wait_mode="sem-ge-imm", wait_value=val, wait_reg=None))
            ins.sync_info = bass_rust.SyncInfo(
                on_wait=waits, on_update=si.on_update)
            continue
        if si is None:
            continue
        if ins.name in stage_writers:
            keep = [w for w in si.on_wait
                    if not ((w.ant_name or "").startswith("DMASW")
                            and w.id not in updaters)]
            ins.sync_info = bass_rust.SyncInfo(
                on_wait=keep, on_update=si.on_update)
            continue
        dead = [w for w in si.on_wait
                if (w.ant_name or "").startswith("DMASW")
                and w.id not in updaters]
        if not dead:
            continue
        new_waits = [w for w in si.on_wait if w not in dead]
        ins.sync_info = bass_rust.SyncInfo(
            on_wait=new_waits, on_update=si.on_update)

    # program completion still gates on the writeback: the final
    # instruction waits the descriptor's completion sem.
    last = all_ins[-1]
    si = last.sync_info or bass_rust.SyncInfo(on_wait=[], on_update=[])
    waits = list(si.on_wait)
    waits.append(bass_rust.SyncWait(
        sync_type="semaphore", id=my_sem_id, ant_name="swdge_out",
        wait_mode="sem-ge-imm", wait_value=16, wait_reg=None))
    last.sync_info = bass_rust.SyncInfo(on_wait=waits, on_update=si.on_update)


def _build_program():
    import concourse.bass as bass
    import concourse.bacc as bacc
    import concourse.mybir as mybir
    from concourse.mybir import AluOpType
    from concourse.tile import TileContext
    from contextlib import ExitStack

    f32 = mybir.dt.float32
    f16 = mybir.dt.float16
    i32 = mybir.dt.int32
    AF = mybir.ActivationFunctionType

    NDVE = CB - NPOOL

    nc = bacc.Bacc("TRN2", debug=False)

    # dl: durs | logh (f32); ee: edges | evs (f16)
    dl = nc.dram_tensor("dl", [P, 2 * CB], f32, kind="ExternalInput").ap()
    ee = nc.dram_tensor("ee", [P, NB + CB], f16, kind="ExternalInput").ap()
    out = nc.dram_tensor("out", [1, P, 1, 64], f32, kind="ExternalOutput").ap()

    with TileContext(nc) as tc, ExitStack() as ctx:
        sg = ctx.enter_context(tc.tile_pool(name="sg", bufs=1))
        dve_pool = ctx.enter_context(tc.tile_pool(name="dve_pool", bufs=NDVE))
        gp_pool = ctx.enter_context(tc.tile_pool(name="gp_pool", bufs=max(NPOOL, 1)))
        psums = ctx.enter_context(tc.tile_pool(name="psums", bufs=1, space="PSUM"))

        # ---- early, data-independent work ----
        eps_sb = sg.tile([P, 1], f32, tag="eps_sb")
        nc.vector.memset(eps_sb[:], EPS)
        onesF = sg.tile([P, 1], f32, tag="onesF")
        nc.vector.memset(onesF[:], 1.0)
        st_tile = sg.tile([P, 1, 1, 64], f32, tag="out_sb")
        stage = st_tile[:]
        evones = sg.tile([P, 2, CB], f16, tag="evones")
        nc.vector.memset(evones[:, 1, :], 1.0)
        # fire the activation-table load (natural_log set: Ln + Square)
        dummy = sg.tile([P, 1], f32, tag="dummy")
        nc.scalar.activation(dummy[:], eps_sb[:], AF.Ln)

        # ---- inputs ----
        dl_sb = sg.tile([P, 2 * CB], f32, tag="dl_sb")
        nc.sync.dma_start(out=dl_sb[:], in_=dl)

        if OUT_MODE == "kvwb":
            ctxz = sg.tile([P, 1], i32, tag="ctxz")
            nc.gpsimd.memset(ctxz[:], 0)
        ee_sb = sg.tile([P, NB + CB], f16, tag="ee_sb")
        nc.gpsimd.dma_start(out=ee_sb[:], in_=ee)

        if OUT_MODE == "kvwb":
            # prep the output-writeback descriptors during the input head;
            # kv_writeback fully overwrites the [128,64] out dram region, so
            # no zero pass is needed.  trigger_dma fires it at the end (the
            # trigger carries the RAW edge on the stage writers).
            dma_sem = nc.alloc_semaphore("swdge_out")
            nc.gpsimd.kv_writeback(
                out, stage, ctxz[:], prepare_only=True, sem=dma_sem
            )

        edges_sb = ee_sb[:, 0:NB]
        evs_sb = ee_sb[:, NB : NB + CB]
        durs_sb = dl_sb[:, 0:CB]
        logh_sb = dl_sb[:, CB : 2 * CB]

        # ---- lnd = ln(durs + EPS); e (for penalties) on Pool ----
        lnd = sg.tile([P, CB], f32, tag="lnd")
        nc.scalar.activation(lnd[:], durs_sb, AF.Ln, bias=eps_sb[:])
        e_sb = sg.tile([P, CB], f32, tag="e_sb")
        nc.gpsimd.tensor_tensor(e_sb[:], lnd[:], logh_sb, AluOpType.subtract)

        # ---- stationary [ev, ones] pairs (moving operand per j-block) ----
        nc.gpsimd.tensor_copy(evones[:, 0, :], evs_sb)

        # ---- penalties: Act Square accum rows -> one tiny f32 matmul
        # (lands in column 2 of the shared psum tile so a single epilogue
        # copy stages everything; issued on PE before the chain).
        pen2 = sg.tile([P, 2], f32, tag="pen2")
        pen_scr = sg.tile([P, CB], f32, tag="pen_scr")
        nc.scalar.activation(pen_scr[:], e_sb[:], AF.Square, accum_out=pen2[:, 0:1])
        nc.scalar.activation(pen_scr[:], logh_sb, AF.Square, accum_out=pen2[:, 1:2])
        psum_m = psums.tile([NB, 3], f32, tag="psum_m")
        nc.tensor.matmul(psum_m[0:2, 2:3], pen2[:], onesF[:], start=True, stop=True)

        # ---- main loop: fused compare g_c[p,k] = 1[t_k + h_p <= lnd_p]
        # (== t_k <= e_p), so compares need only lnd + logh, skipping the
        # subtract on the critical path; g is the matmul stationary.
        pool_pos = set()
        if NPOOL > 0:
            stride = CB / NPOOL
            pool_pos = {min(CB - 2, int(stride * i + 1)) for i in range(NPOOL)}
            while len(pool_pos) < NPOOL:  # collision fallback
                pool_pos.add(max(0, CB - 2 - len(pool_pos)))
        for c in range(CB):
            eng = nc.gpsimd if c in pool_pos else nc.vector
            pool = gp_pool if c in pool_pos else dve_pool
            g = pool.tile([P, NB], f16, tag="g")
            eng.tensor_scalar(
                g[:], edges_sb, logh_sb[:, c : c + 1], lnd[:, c : c + 1],
                AluOpType.add, AluOpType.is_le,
            )
            mov = evones[:, :, c : c + 1].rearrange("p a b -> p (a b)")
            nc.tensor.matmul(
                psum_m[:, 0:2], g[:], mov,
                start=(c == 0), stop=(c == CB - 1),
            )

        # ---- epilogue: stage [D|C|pens], then fire the writeback ----
        w2 = nc.vector.tensor_copy(stage[0:NB, 0, 0, 0:3], psum_m[:])
        stage_writers = [w2.ins.name]
        if OUT_MODE == "kvwb":
            trigger_name = nc.gpsimd.trigger_dma(count=None).ins.name
        else:
            nc.sync.dma_start(
                out=out[0, 0:NB, 0, :], in_=stage[0:NB, 0, 0, :]
            )

    if OUT_MODE == "kvwb":
        _fix_writeback_sync(nc, stage_writers, trigger_name)

    nc.compile()
    return nc


def _get_program():
    key = (NB, NPOOL, OUT_MODE)
    if key not in _prog_cache:
        _prog_cache[key] = _build_program()
    return _prog_cache[key]


def _make_in_maps(log_h, durations, events):
    log_h = np.ascontiguousarray(np.asarray(log_h, dtype=np.float32)).reshape(N)
    durations = np.ascontiguousarray(np.asarray(durations, dtype=np.float32)).reshape(N)
    events = np.ascontiguousarray(np.asarray(events, dtype=np.float32)).reshape(N)

    edges16 = np.empty(NB, dtype=np.float16)
    edges16[0] = SENTINEL
    edges16[1:] = _edges_f64().astype(np.float16)

    SL = N // NCORES
    in_maps = []
    for k in range(NCORES):
        sl = slice(k * SL, (k + 1) * SL)
        dl_np = np.empty((P, 2 * CB), dtype=np.float32)
        dl_np[:, 0:CB] = durations[sl].reshape(P, CB)
        dl_np[:, CB : 2 * CB] = log_h[sl].reshape(P, CB)
        ee_np = np.empty((P, NB + CB), dtype=np.float16)
        ee_np[:, 0:NB] = edges16[None, :]
        ee_np[:, NB : NB + CB] = events[sl].reshape(P, CB).astype(np.float16)
        in_maps.append({"dl": dl_np, "ee": ee_np})
    return in_maps


def kernel(log_h, durations, events):
    global last_results
    from concourse import bass_utils

    nc = _get_program()
    in_maps = _make_in_maps(log_h, durations, events)
    res = bass_utils.run_bass_kernel_spmd(
        nc, in_maps, core_ids=list(range(NCORES))
    )
    last_results = res

    D = np.zeros(NB, dtype=np.float64)
    C = np.zeros(NB, dtype=np.float64)
    e2 = 0.0
    lh2 = 0.0
    for k in range(NCORES):
        A = np.asarray(res.results[k]["out"], dtype=np.float64).reshape(P, 64)[:NB]
        D += A[:, 0]
        C += A[:, 1]
        e2 += A[0, 2]
        lh2 += A[1, 2]

    Ev = D[0]  # sentinel column: every e >= -6e4
    w = _edge_weights()
    pair = float(np.sum(w * C * (Ev - D)))
    loss = pair / float(N) ** 2 + ALPHA * e2 / N + BETA * lh2 / N
    return np.float32(loss)


# revision 14
# speedup vs baseline: 1.0868x; 1.0115x over previous
"""DSAFT rank-loss kernel for 8 Trainium2 NeuronCores (Bass/Tile).

loss = (1/n^2) * sum_{i,j} relu(e_j - e_i) * events_i
       + ALPHA * sum(e^2)/n + BETA * sum(log_h^2)/n
with e = log(durations + EPS) - log_h, n = 16384.

Algorithm (quantized staircase, O(n*B)):
  For an increasing edge grid t_1..t_B with per-edge weights w_k
  (midpoint gaps), relu(e_j - e_i) ~= sum_k w_k 1[e_i < t_k <= e_j], so
     pair ~= sum_k w_k * C_k * (Ev - D_k)
  with C_k = #{j : e_j >= t_k}, D_k = sum_i ev_i 1[e_i >= t_k],
  Ev = sum_i ev_i.  B=63 edges + one sentinel edge at -6e4 whose D
  column recovers Ev (and whose weight is 0).  Offline validation:
  rel err <= 1.8e-3 worst-of-13 draws (gate 2e-2).

Sharding: rows (j) are split across the 8 cores -- each core holds its
2048 elements (16 j-blocks of 128) and the full 64-slot edge vector,
computes partial C/D via PE, and the host sums the 8 partial [64,2]
vectors before the final O(B) combine.

Per-core pipeline (TimelineSim-costed):
  - head (~2.3us fixed): dl=[durs|logh] f32 via SP HWDGE DMA;
    ee=[edges|evs] f16 via Pool SWDGE DMA; act-table preload, memsets,
    scatter-index iota, and an out-zeroing DMA all overlap the head.
  - e = Ln(durs+EPS) - logh (Act then DVE).
  - 16 compare tiles g_c[p,k] = (t_k <= e_p) f16, split DVE(12)/Pool(4)
    (DVE 4x mode ~77ns/tile).  Each g is the matmul STATIONARY;
    moving is [ev_c, ones] [128,2], accumulating psum[64,2] = [D|C]
    per edge -- PE cost is 2 cycles/block (cost = moving free size).
  - penalties via Act Square accum rows + one tiny f32 matmul.
  - epilogue: 2 PSUM->SBUF copies, then a PREPARED SWDGE scatter-add
    fires via trigger_dma (tail ~1.0us instead of ~2.2us HWDGE).
"""

import os

import numpy as np

N = 16384
P = 128            # partitions / j's per block
CB = 16            # j-blocks per core (N / NCORES / P)
NCORES = 8
ALPHA = 0.001
BETA = 0.001
EPS = 1e-32

# staircase quantization: NB slots = 1 sentinel + B_REAL real edges
NB = int(os.environ.get("KERN_NB", "64"))
B_REAL = NB - 1
T0 = -16.0
T1 = 6.0
SENTINEL = -60000.0

# Tuning knobs
NPOOL = int(os.environ.get("KERN_NPOOL", "4"))   # j-blocks on the Pool stream
OUT_MODE = os.environ.get("KERN_OUT", "kvwb")  # kvwb | plain

_prog_cache = {}
last_results = None  # BassKernelResults of the most recent run (for profiling)


def _edges_f64():
    """Real edge positions: emulate the device's f32 affine iota -> f16."""
    w = np.float32((T1 - T0) / B_REAL)
    b = np.float32(T0 - 0.5 * float(w))
    k = np.arange(1, B_REAL + 1, dtype=np.float32)
    t = (k * w + b).astype(np.float16)
    return t.astype(np.float64)


def _edge_weights():
    """Host-side per-slot weights: w[0]=0 (sentinel), midpoint gaps else."""
    t = _edges_f64()
    w = np.empty(NB, dtype=np.float64)
    w[0] = 0.0
    wr = np.empty(B_REAL, dtype=np.float64)
    if B_REAL > 1:
        wr[1:-1] = (t[2:] - t[:-2]) / 2.0
        wr[0] = t[1] - t[0]
        wr[-1] = t[-1] - t[-2]
    else:
        wr[0] = (T1 - T0)
    w[1:] = wr
    return w


def _fix_writeback_sync(nc, stage_writers, trigger_name):
    """Re-plumb the PREPARE_ONLY writeback's synchronization.

    Tile's model for a prepared SWDGE writeback assumes double-buffering:
    writers of the prep's source that come after the prep must wait for the
    DMA (a WAR wait on the prep's DMASW lane), and the trigger is unordered
    against them.  This kernel fills the staging tile after the prep and
    fires the trigger last, so that model (a) deadlocks -- the lane sem is
    never incremented for prepared DMAs -- and (b) leaves the trigger racing
    the stage writers on real hardware.  Rewrite at the BIR level, with the
    same sem encoding Tile itself emits:

      1. strip the dead DMASW-lane WAR waits from the stage writers;
      2. make the trigger wait on each stage writer's engine-lane sem at
         its absolute post-write count (write -> DMA read ordering);
      3. repoint remaining waits on the dead lane sem (end-of-program
         drains) at the descriptor's real completion sem (swdge_out >= 16).
    """
    import bass_rust

    all_ins = [i for bb in nc.m.functions[0].blocks for i in bb.instructions]
    my_sem_id = None
    updaters = set()
    for ins in all_ins:
        si = ins.sync_info
        if si is None:
            continue
        for u in si.on_update:
            updaters.add(u.id)
            if (u.ant_name or "") == "swdge_out":
                my_sem_id = u.id
    assert my_sem_id is not None

    # absolute lane-sem value after each stage writer completes
    sem_counts = {}
    writer_waits = []
    for ins in all_ins:
        si = ins.sync_info
        if si is None:
            continue
        for u in si.on_update:
            if u.update_mode == "sem-inc" and isinstance(u.update_value, int):
                sem_counts[u.id] = sem_counts.get(u.id, 0) + u.update_value
                if ins.name in stage_writers:
                    writer_waits.append(
                        (u.id, u.ant_name, sem_counts[u.id]))
    assert len(writer_waits) >= len(stage_writers)

    for ins in all_ins:
        si = ins.sync_info
        if ins.name == trigger_name:
            si = si or bass_rust.SyncInfo(on_wait=[], on_update=[])
            waits = list(si.on_wait)
            for sid, sname, val in writer_waits:
                waits.append(bass_rust.SyncWait(
                    sync_type="semaphore", id=sid, ant_name=sname,
                    wait_mode="sem-ge-imm", wait_value=val, wait_reg=None))
            ins.sync_info = bass_rust.SyncInfo(
                on_wait=waits, on_update=si.on_update)
            continue
        if si is None:
            continue
        if ins.name in stage_writers:
            keep = [w for w in si.on_wait
                    if not ((w.ant_name or "").startswith("DMASW")
                            and w.id not in updaters)]
            ins.sync_info = bass_rust.SyncInfo(
                on_wait=keep, on_update=si.on_update)
            continue
        dead = [w for w in si.on_wait
                if (w.ant_name or "").startswith("DMASW")
                and w.id not in updaters]
        if not dead:
            continue
        new_waits = [w for w in si.on_wait if w not in dead]
        ins.sync_info = bass_rust.SyncInfo(
            on_wait=new_waits, on_update=si.on_update)

    # program completion still gates on the writeback: the final
    # instruction waits the descriptor's completion sem.
    last = all_ins[-1]
    si = last.sync_info or bass_rust.SyncInfo(on_wait=[], on_update=[])
    waits = list(si.on_wait)
    waits.append(bass_rust.SyncWait(
        sync_type="semaphore", id=my_sem_id, ant_name="swdge_out",
        wait_mode="sem-ge-imm", wait_value=16, wait_reg=None))
    last.sync_info = bass_rust.SyncInfo(on_wait=waits, on_update=si.on_update)


def _build_program():
    import concourse.bass as bass
    import concourse.bacc as bacc
    import concourse.mybir as mybir
    from concourse.mybir import AluOpType
    from concourse.tile import TileContext
    from contextlib import ExitStack

    f32 = mybir.dt.float32
    f16 = mybir.dt.float16
    i16 = mybir.dt.int16
    i32 = mybir.dt.int32
    AF = mybir.ActivationFunctionType

    NDVE = CB - NPOOL

    nc = bacc.Bacc("TRN2", debug=False)

    # dl: durs | logh (f32); ee: edges | evs (f16)
    dl = nc.dram_tensor("dl", [P, 2 * CB], f32, kind="ExternalInput").ap()
    ee = nc.dram_tensor("ee", [P, CB], f16, kind="ExternalInput").ap()
    out = nc.dram_tensor("out", [1, P, 1, 64], f32, kind="ExternalOutput").ap()

    with TileContext(nc) as tc, ExitStack() as ctx:
        sg = ctx.enter_context(tc.tile_pool(name="sg", bufs=1))
        dve_pool = ctx.enter_context(tc.tile_pool(name="dve_pool", bufs=NDVE))
        gp_pool = ctx.enter_context(tc.tile_pool(name="gp_pool", bufs=max(NPOOL, 1)))
        psums = ctx.enter_context(tc.tile_pool(name="psums", bufs=1, space="PSUM"))

        # ---- early, data-independent work ----
        eps_sb = sg.tile([P, 1], f32, tag="eps_sb")
        nc.vector.memset(eps_sb[:], EPS)
        onesF = sg.tile([P, 1], f32, tag="onesF")
        nc.vector.memset(onesF[:], 1.0)
        st_tile = sg.tile([P, 1, 1, 64], f32, tag="out_sb")
        stage = st_tile[:]
        evones = sg.tile([P, 2, CB], f16, tag="evones")
        nc.vector.memset(evones[:, 1, :], 1.0)
        # fire the activation-table load (natural_log set: Ln + Square)
        dummy = sg.tile([P, 1], f32, tag="dummy")
        nc.scalar.activation(dummy[:], eps_sb[:], AF.Ln)

        # ---- inputs ----
        dl_sb = sg.tile([P, 2 * CB], f32, tag="dl_sb")
        nc.sync.dma_start(out=dl_sb[:], in_=dl)

        # edges are data-independent: generate them on-device during the
        # DMA head (iota + affine convert + sentinel memset) so the compare
        # stream is gated only by the Ln chain, not an edge DMA.
        edges_i = sg.tile([P, NB], i32, tag="edges_i")
        nc.gpsimd.iota(edges_i[:], pattern=[[1, NB]], base=0,
                       channel_multiplier=0)
        edges_sb = sg.tile([P, NB], f16, tag="edges_sb")
        W_EDGE = (T1 - T0) / B_REAL
        nc.vector.tensor_scalar(
            edges_sb[:], edges_i[:], float(np.float32(W_EDGE)),
            float(np.float32(T0 - 0.5 * W_EDGE)),
            AluOpType.mult, AluOpType.add,
        )
        nc.vector.memset(edges_sb[:, 0:1], SENTINEL)
        ee_sb = sg.tile([P, CB], f16, tag="ee_sb")
        nc.gpsimd.dma_start(out=ee_sb[:], in_=ee)

        if OUT_MODE == "kvwb":
            # prep the output-writeback descriptors during the input head;
            # kv_writeback fully overwrites the [128,64] out dram region, so
            # no zero pass is needed.  trigger_dma fires it at the end (the
            # trigger carries the RAW edge on the stage writers).
            ctxz = sg.tile([P, 1], i32, tag="ctxz")
            nc.gpsimd.memset(ctxz[:], 0)
            dma_sem = nc.alloc_semaphore("swdge_out")
            nc.gpsimd.kv_writeback(
                out, stage, ctxz[:], prepare_only=True, sem=dma_sem
            )

        evs_sb = ee_sb[:]
        durs_sb = dl_sb[:, 0:CB]
        logh_sb = dl_sb[:, CB : 2 * CB]

        # ---- lnd = ln(durs + EPS); e (for penalties) on Pool ----
        lnd = sg.tile([P, CB], f32, tag="lnd")
        nc.scalar.activation(lnd[:], durs_sb, AF.Ln, bias=eps_sb[:])
        e_sb = sg.tile([P, CB], f32, tag="e_sb")
        nc.gpsimd.tensor_tensor(e_sb[:], lnd[:], logh_sb, AluOpType.subtract)

        # ---- stationary [ev, ones] pairs (moving operand per j-block) ----
        nc.gpsimd.tensor_copy(evones[:, 0, :], evs_sb)

        # ---- penalties: Act Square accum rows -> one tiny f32 matmul
        # (lands in column 2 of the shared psum tile so a single epilogue
        # copy stages everything; issued on PE before the chain).
        pen2 = sg.tile([P, 2], f32, tag="pen2")
        pen_scr = sg.tile([P, CB], f32, tag="pen_scr")
        nc.scalar.activation(pen_scr[:], e_sb[:], AF.Square, accum_out=pen2[:, 0:1])
        nc.scalar.activation(pen_scr[:], logh_sb, AF.Square, accum_out=pen2[:, 1:2])
        psum_m = psums.tile([NB, 3], f32, tag="psum_m")
        nc.tensor.matmul(psum_m[0:2, 2:3], pen2[:], onesF[:], start=True, stop=True)

        # ---- main loop: fused compare g_c[p,k] = 1[t_k + h_p <= lnd_p]
        # (== t_k <= e_p), so compares need only lnd + logh, skipping the
        # subtract on the critical path; g is the matmul stationary.
        pool_pos = set()
        if NPOOL > 0:
            stride = CB / NPOOL
            pool_pos = {min(CB - 2, int(stride * i + 1)) for i in range(NPOOL)}
            while len(pool_pos) < NPOOL:  # collision fallback
                pool_pos.add(max(0, CB - 2 - len(pool_pos)))
        for c in range(CB):
            eng = nc.gpsimd if c in pool_pos else nc.vector
            pool = gp_pool if c in pool_pos else dve_pool
            g = pool.tile([P, NB], f16, tag="g")
            eng.tensor_scalar(
                g[:], edges_sb, logh_sb[:, c : c + 1], lnd[:, c : c + 1],
                AluOpType.add, AluOpType.is_le,
            )
            mov = evones[:, :, c : c + 1].rearrange("p a b -> p (a b)")
            nc.tensor.matmul(
                psum_m[:, 0:2], g[:], mov,
                start=(c == 0), stop=(c == CB - 1),
            )

        # ---- epilogue: stage [D|C|pens], then fire the writeback ----
        w2 = nc.vector.tensor_copy(stage[0:NB, 0, 0, 0:3], psum_m[:])
        stage_writers = [w2.ins.name]
        if OUT_MODE == "kvwb":
            trigger_name = nc.gpsimd.trigger_dma(count=None).ins.name
        else:
            nc.sync.dma_start(
                out=out[0, 0:NB, 0, :], in_=stage[0:NB, 0, 0, :]
            )

    if OUT_MODE == "kvwb":
        _fix_writeback_sync(nc, stage_writers, trigger_name)

    nc.compile()
    return nc


def _get_program():
    key = (NB, NPOOL, OUT_MODE)
    if key not in _prog_cache:
        _prog_cache[key] = _build_program()
    return _prog_cache[key]


def _make_in_maps(log_h, durations, events):
    log_h = np.ascontiguousarray(np.asarray(log_h, dtype=np.float32)).reshape(N)
    durations = np.ascontiguousarray(np.asarray(durations, dtype=np.float32)).reshape(N)
    events = np.ascontiguousarray(np.asarray(events, dtype=np.float32)).reshape(N)

    SL = N // NCORES
    in_maps = []
    for k in range(NCORES):
        sl = slice(k * SL, (k + 1) * SL)
        dl_np = np.empty((P, 2 * CB), dtype=np.float32)
        dl_np[:, 0:CB] = durations[sl].reshape(P, CB)
        dl_np[:, CB : 2 * CB] = log_h[sl].reshape(P, CB)
        ee_np = np.ascontiguousarray(
            events[sl].reshape(P, CB).astype(np.float16))
        in_maps.append({"dl": dl_np, "ee": ee_np})
    return in_maps


def kernel(log_h, durations, events):
    global last_results
    from concourse import bass_utils

    nc = _get_program()
    in_maps = _make_in_maps(log_h, durations, events)
    res = bass_utils.run_bass_kernel_spmd(
        nc, in_maps, core_ids=list(range(NCORES))
    )
    last_results = res

    D = np.zeros(NB, dtype=np.float64)
    C = np.zeros(NB, dtype=np.float64)
    e2 = 0.0
    lh2 = 0.0
    for k in range(NCORES):
        A = np.asarray(res.results[k]["out"], dtype=np.float64).reshape(P, 64)[:NB]
        D += A[:, 0]
        C += A[:, 1]
        e2 += A[0, 2]
        lh2 += A[1, 2]

    Ev = D[0]  # sentinel column: every e >= -6e4
    w = _edge_weights()
    pair = float(np.sum(w * C * (Ev - D)))
    loss = pair / float(N) ** 2 + ALPHA * e2 / N + BETA * lh2 / N
    return np.float32(loss)
